# revision 1
# baseline (speedup 1.0000x reference)
"""Distributed causal multi-head attention (Bass/Tile, 8 TRN2 NeuronCores).

Sharding: core = (batch b, rank r) with b = core // 4, r = core % 4.
Within a batch group of 4 cores, rank r owns query rows {g : g % 4 == r}
(row-interleaved sequence parallelism).  Every core runs the IDENTICAL
graph; rank-dependence lives entirely in the input data (x^T shard and a
small diagonal-mask tensor built on the host).

Per core:
  q^T, k^T = (x_own @ Wq/Wk)^T   [C, 512]   (transposed orientation)
  v        =  x_own @ Wv         [512, C]   (normal orientation)
  AllGather (bf16) of packed [k^T | v] across the 4 ranks
  scores[tq, tk] = q^T.T @ k^T chunks  (keys in rank-permuted order)
  softmax: fused exp + row-sum via activation(accum_out), per-partition
  normalize, PE-transpose att tiles, AV matmul (2 heads col-packed)
  producing out^T directly, then y = out^T.T @ Wo.
"""

import numpy as np

B, T, C, H = 2, 2048, 1024, 16
D = C // H            # 64
R = 4                 # ranks per batch group
TOWN = T // R         # 512 rows owned per core
NJ = T // 512         # 4 key 512-chunks
NT = TOWN // 128      # 4 local query 128-tiles
CC = C // 128         # 8 contraction chunks
PAIRS = H // 2        # 8 head pairs
KT_ELEMS = C * TOWN   # k^T shard elems
V_ELEMS = TOWN * C    # v shard elems
AG_ELEMS = KT_ELEMS + V_ELEMS
SCALE = 1.0 / 32.0    # 1/sqrt(C)
NEG = -1e30

_cached_nc = None
last_result = None
_DEBUG = False


def _dbg(nc, P, col, ap, width):
    if P.get("dbg_ext") is not None:
        nc.sync.dma_start(P["dbg_ext"][:, col : col + width], ap)


def _qkv_phase(nc, P, mybir):
    """k^T and v in halves, four quarter-size AllGathers interleaved, then q^T."""
    F32, BF16 = mybir.dt.float32, mybir.dt.bfloat16
    wqkv_sb, xt_sb = P["wqkv_sb"], P["xt_sb"]
    mm_p = P["mm_p"]
    groups = [[0, 1, 2, 3], [4, 5, 6, 7]]
    VW = PAIRS * 130  # 1040
    HKT = 4 * TOWN * 128  # elems per half k^T bounce

    kt_loc = P["kv_p"].tile([128, CC * TOWN], BF16, tag="kt_loc")
    v_loc = P["kv_p"].tile([128, NT * VW], BF16, tag="v_loc")
    nc.vector.memset(
        v_loc[:].rearrange("p (ch x) -> p ch x", x=65)[:, :, 64:65], 1.0
    )

    def kt_half(half):
        for kc in range(4 * half, 4 * (half + 1)):
            ps = mm_p.tile([128, 512], F32, tag="sc")
            for cc in range(CC):
                nc.tensor.matmul(
                    ps[:],
                    wqkv_sb[:, cc * 3 * C + C + kc * 128 : cc * 3 * C + C + (kc + 1) * 128],
                    xt_sb[:, cc * TOWN : (cc + 1) * TOWN],
                    start=(cc == 0),
                    stop=(cc == CC - 1),
                )
            nc.vector.tensor_copy(kt_loc[:, kc * TOWN : (kc + 1) * TOWN], ps[:])
        bounce = P["dram_p"].tile([HKT], BF16, tag=f"bounce_kt{half}")
        for i, kc in enumerate(range(4 * half, 4 * (half + 1))):
            nc.sync.dma_start(
                bounce[i * TOWN * 128 : (i + 1) * TOWN * 128].rearrange("(p f) -> p f", p=128),
                kt_loc[:, kc * TOWN : (kc + 1) * TOWN],
            )
        gath = P["dram_p"].tile([R * HKT], BF16, tag=f"gathered_kt{half}")
        nc.gpsimd.collective_compute(
            "AllGather", mybir.AluOpType.bypass, replica_groups=groups,
            ins=[bounce.opt()], outs=[gath.opt()],
        )
        P[f"gathered_kt{half}"] = gath

    def v_half(hf):
        for t in range(NT):
            ps = mm_p.tile([128, 512], F32, tag="sc")
            for cc in range(CC):
                nc.tensor.matmul(
                    ps[:],
                    xt_sb[:, cc * TOWN + t * 128 : cc * TOWN + (t + 1) * 128],
                    wqkv_sb[:, cc * 3 * C + 2 * C + hf * 512 : cc * 3 * C + 2 * C + (hf + 1) * 512],
                    start=(cc == 0),
                    stop=(cc == CC - 1),
                )
            for hh in range(2):
                nc.vector.tensor_copy(
                    v_loc[:, t * VW + 4 * hf * 130 : t * VW + 4 * (hf + 1) * 130].rearrange(
                        "p (pr x) -> p pr x", x=130
                    )[:, :, hh * 65 : hh * 65 + 64],
                    ps[:].rearrange("p (pr hc) -> p pr hc", hc=128)[:, :, hh * 64 : (hh + 1) * 64],
                )
        bounce = P["dram_p"].tile([NT * 128 * 520], BF16, tag=f"bounce_v{hf}")
        for t in range(NT):
            nc.sync.dma_start(
                bounce[t * 520 * 128 : (t + 1) * 520 * 128].rearrange("(p f) -> p f", p=128),
                v_loc[:, t * VW + 4 * hf * 130 : t * VW + 4 * (hf + 1) * 130],
            )
        gath = P["dram_p"].tile([R * NT * 128 * 520], BF16, tag=f"gathered_v{hf}")
        nc.gpsimd.collective_compute(
            "AllGather", mybir.AluOpType.bypass, replica_groups=groups,
            ins=[bounce.opt()], outs=[gath.opt()],
        )
        P[f"gathered_v{hf}"] = gath

    kt_half(0)
    v_half(0)
    kt_half(1)
    v_half(1)

    qt_sb = P["qt_p"].tile([128, CC * TOWN], BF16, tag="qt")
    for qc in range(CC):
        ps = mm_p.tile([128, 512], F32, tag="sc")
        for cc in range(CC):
            nc.tensor.matmul(
                ps[:],
                wqkv_sb[:, cc * 3 * C + qc * 128 : cc * 3 * C + (qc + 1) * 128],
                xt_sb[:, cc * TOWN : (cc + 1) * TOWN],
                start=(cc == 0),
                stop=(cc == CC - 1),
            )
        nc.vector.tensor_scalar_mul(qt_sb[:, qc * TOWN : (qc + 1) * TOWN], ps[:], SCALE)
    P["qt_sb"] = qt_sb


def _gather_kv_pair(nc, P, p, mybir):
    """Load this head pair's gathered k^T and v into SBUF (rank-major cols)."""
    BF16 = mybir.dt.bfloat16
    half, pl = p // 4, p % 4
    gkt = P[f"gathered_kt{half}"]
    gv = P[f"gathered_v{half}"]
    HKT = 4 * TOWN * 128
    ktg = P["ktg_p"].tile([128, 2048], BF16, tag="ktg")
    for s in range(R):
        src = gkt[
            s * HKT + pl * 128 * TOWN : s * HKT + (pl + 1) * 128 * TOWN
        ].rearrange("(q f) -> q f", q=128)
        nc.sync.dma_start(ktg[:, s * 512 : (s + 1) * 512], src)
    vg = P["vg_p"].tile([128, 2080], BF16, tag="vg")
    for s in range(R):
        src = gv[s * NT * 128 * 520 : (s + 1) * NT * 128 * 520].rearrange(
            "(j i x) -> i j x", i=128, x=520
        )[:, :, pl * 130 : (pl + 1) * 130]
        nc.sync.dma_start(
            vg[:, s * 520 : (s + 1) * 520].rearrange("i (j x) -> i j x", x=130), src
        )
    return ktg, vg


def _attention_pair2(nc, P, pA, ktgA, vgA, pB, ktgB, vgB, mybir):
    """Scores^T + exp + AV for two head pairs, j-loops interleaved."""
    F32, BF16 = mybir.dt.float32, mybir.dt.bfloat16
    AFT = mybir.ActivationFunctionType
    qt_sb, dmask = P["qt_sb"], P["dmask"]
    mm_p, attT_p, sm_p = P["mm_p"], P["attT_p"], P["sm_p"]

    avsP = {}
    for p in (pA, pB):
        a0 = P["av_p"].tile([65, TOWN], F32, tag="av")
        a1 = P["av_p"].tile([65, TOWN], F32, tag="av")
        avsP[p] = [a0, a1]
    first = {pA: [True, True], pB: [True, True]}
    for j in range(16):
        jj, sb = j // 4, j % 4
        l0 = jj * 128
        kcol = sb * 512 + jj * 128
        vcol = (sb * 4 + jj) * 130
        for p, ktg, vg in ((pA, ktgA, vgA), (pB, ktgB, vgB)):
            avs = avsP[p]
            for hh in range(2):
                ps = mm_p.tile([128, 512], F32, tag="sc")
                nc.tensor.matmul(
                    ps[:, l0:],
                    ktg[hh * 64 : (hh + 1) * 64, kcol : kcol + 128],
                    qt_sb[hh * 64 : (hh + 1) * 64, p * TOWN + l0 : (p + 1) * TOWN],
                    start=True,
                    stop=True,
                )
                nc.vector.tensor_add(
                    ps[:, l0 : l0 + 128],
                    ps[:, l0 : l0 + 128],
                    dmask[:, sb * 256 + hh * 128 : sb * 256 + (hh + 1) * 128],
                )
                att2 = attT_p.tile([128, 512], BF16, tag="attT")
                nc.scalar.activation(att2[:, l0:], ps[:, l0:], AFT.Exp)
                nc.tensor.matmul(
                    avs[hh][:, l0:],
                    vg[:, vcol + hh * 65 : vcol + (hh + 1) * 65],
                    att2[:, l0:],
                    start=first[p][hh],
                    stop=(j == 15),
                )
                first[p][hh] = False

    # unnormalized out^T and denominator rows; normalization is deferred
    for p in (pA, pB):
        avs = avsP[p]
        den_st = P["sm_p"].tile([128, 2 * TOWN], F32, tag="den_st")
        for hh in range(2):
            nc.vector.tensor_copy(
                P["outT_sb"][hh * 64 : (hh + 1) * 64, p * TOWN : (p + 1) * TOWN],
                avs[hh][0:64, :],
            )
            nc.vector.tensor_copy(
                den_st[64:65, hh * TOWN : (hh + 1) * TOWN], avs[hh][64:65, :]
            )
        dr = 32 * (p // 2) + 2 * (p % 2)
        nc.sync.dma_start(P["den_mat"][dr : dr + 2, :], den_st[64:65, :])


def _normalize_pg(nc, P, pg, mybir):
    """Normalize the four heads of one pair-group (overlaps later groups)."""
    F32, BF16 = mybir.dt.float32, mybir.dt.bfloat16
    den_mat, outT_sb = P["den_mat"], P["outT_sb"]
    r0 = 32 * pg
    nc.vector.reciprocal(den_mat[r0 : r0 + 4, :], den_mat[r0 : r0 + 4, :])
    recb4 = P["sm_p"].tile([128, TOWN], BF16, tag="recb4")
    nc.vector.tensor_copy(recb4[r0 : r0 + 4, :], den_mat[r0 : r0 + 4, :])
    for hi in range(4):
        h = 4 * pg + hi
        lo = (h % 2) * 64
        recb = P["sm_p"].tile([1, TOWN], BF16, tag="recb")
        nc.sync.dma_start(recb[:], recb4[r0 + hi : r0 + hi + 1, :])
        bc = P["av_p"].tile([128, TOWN], F32, tag="av")
        nc.tensor.matmul(
            bc[lo : lo + 64, :], P["ones64"][:], recb[:],
            start=True, stop=True, tile_position=(0, lo),
        )
        bcs = P["sm_p"].tile([128, TOWN], BF16, tag="bcs")
        nc.vector.tensor_copy(bcs[lo : lo + 64, :], bc[lo : lo + 64, :])
        nc.vector.tensor_mul(
            outT_sb[lo : lo + 64, (h // 2) * TOWN : (h // 2 + 1) * TOWN],
            outT_sb[lo : lo + 64, (h // 2) * TOWN : (h // 2 + 1) * TOWN],
            bcs[lo : lo + 64, :],
        )


def _wo_phase(nc, P, mybir):
    from concourse.bass import ts
    F32 = mybir.dt.float32
    BF16 = mybir.dt.bfloat16
    wo_sb = P["w_p"].tile([128, CC * C], BF16, tag="wo")
    for cc in range(CC):
        nc.sync.dma_start(wo_sb[:, cc * C : (cc + 1) * C], P["wo_ext"][ts(cc, 128), :])
    outT_sb, mm_p = P["outT_sb"], P["mm_p"]
    y_sb = P["y_p"].tile([128, NT * C], F32, tag="y")
    for t in range(NT):
        for hf in range(2):
            ps = mm_p.tile([128, 512], F32, tag="sc")
            for cc in range(CC):
                nc.tensor.matmul(
                    ps[:],
                    outT_sb[:, cc * TOWN + t * 128 : cc * TOWN + (t + 1) * 128],
                    wo_sb[:, cc * C + hf * 512 : cc * C + (hf + 1) * 512],
                    start=(cc == 0),
                    stop=(cc == CC - 1),
                )
            nc.vector.tensor_copy(y_sb[:, t * C + hf * 512 : t * C + (hf + 1) * 512], ps[:])
    for t in range(NT):
        nc.sync.dma_start(P["out_ext"][t * 128 : (t + 1) * 128, :], y_sb[:, t * C : (t + 1) * C])


def _body(nc, P, mybir):
    from concourse.bass import ts

    F32, BF16 = mybir.dt.float32, mybir.dt.bfloat16

    ones64 = P["const_p"].tile([1, 64], BF16, tag="ones64")
    nc.vector.memset(ones64[:], 1.0)
    P["ones64"] = ones64
    dmask = P["const_p"].tile([128, 1024], F32, tag="dmask")
    nc.sync.dma_start(dmask[:], P["dmask_ext"][:])
    P["dmask"] = dmask

    xt_sb = P["x_p"].tile([128, CC * TOWN], BF16, tag="xt")
    for cc in range(CC):
        nc.sync.dma_start(xt_sb[:, cc * TOWN : (cc + 1) * TOWN], P["xt_ext"][ts(cc, 128), :])
    P["xt_sb"] = xt_sb
    wqkv_sb = P["w_p"].tile([128, CC * 3 * C], BF16, tag="wqkv")
    for part in (1, 2, 0):  # k first (feeds the AllGather), then v, then q
        for cc in range(CC):
            nc.sync.dma_start(
                wqkv_sb[:, cc * 3 * C + part * C : cc * 3 * C + (part + 1) * C],
                P["wqkv_ext"][ts(cc, 128), part * C : (part + 1) * C],
            )
    P["wqkv_sb"] = wqkv_sb

    _qkv_phase(nc, P, mybir)

    outT_sb = P["outT_p"].tile([128, PAIRS * TOWN], BF16, tag="outT")
    P["outT_sb"] = outT_sb
    den_mat = P["sm_p"].tile([128, TOWN], F32, tag="den_mat")
    P["den_mat"] = den_mat
    for pg in range(PAIRS // 2):
        pA, pB = 2 * pg, 2 * pg + 1
        ktgA, vgA = _gather_kv_pair(nc, P, pA, mybir)
        ktgB, vgB = _gather_kv_pair(nc, P, pB, mybir)
        _attention_pair2(nc, P, pA, ktgA, vgA, pB, ktgB, vgB, mybir)
        _normalize_pg(nc, P, pg, mybir)

    _wo_phase(nc, P, mybir)


def _build():
    import concourse.mybir as mybir
    import concourse.tile as tile
    from concourse import bacc

    F32, BF16 = mybir.dt.float32, mybir.dt.bfloat16

    nc = bacc.Bacc("TRN2", target_bir_lowering=False, debug=False, num_devices=8)
    P = {
        "xt_ext": nc.declare_dram_parameter("xt", [C, TOWN], BF16, isOutput=False),
        "wqkv_ext": nc.declare_dram_parameter("wqkv", [C, 3 * C], BF16, isOutput=False),
        "wo_ext": nc.declare_dram_parameter("wo", [C, C], BF16, isOutput=False),
        "dmask_ext": nc.declare_dram_parameter("dmask", [128, 1024], F32, isOutput=False),
        "out_ext": nc.declare_dram_parameter("out", [TOWN, C], F32, isOutput=True),
    }
    if _DEBUG:
        P["dbg_ext"] = nc.declare_dram_parameter("dbg", [128, 10240], BF16, isOutput=True)

    with tile.TileContext(nc) as tc:
        with (
            tc.tile_pool(name="const", bufs=1) as const_p,
            tc.tile_pool(name="w", bufs=1) as w_p,
            tc.tile_pool(name="x", bufs=1) as x_p,
            tc.tile_pool(name="qt", bufs=1) as qt_p,
            tc.tile_pool(name="kv", bufs=1) as kv_p,
            tc.tile_pool(name="ktg", bufs=4) as ktg_p,
            tc.tile_pool(name="vg", bufs=4) as vg_p,
            tc.tile_pool(name="attT", bufs=8) as attT_p,
            tc.tile_pool(name="outT", bufs=1) as outT_p,
            tc.tile_pool(name="y", bufs=1) as y_p,
            tc.tile_pool(name="sm", bufs=2) as sm_p,
            tc.tile_pool(name="mmp", bufs=4, space="PSUM") as mm_p,
            tc.tile_pool(name="avp", bufs=4, space="PSUM") as av_p,
            tc.tile_pool(name="dram", bufs=1, space="DRAM") as dram_p,
        ):
            P.update(
                const_p=const_p, w_p=w_p, x_p=x_p, qt_p=qt_p, kv_p=kv_p,
                ktg_p=ktg_p, vg_p=vg_p, attT_p=attT_p,
                outT_p=outT_p, y_p=y_p, sm_p=sm_p, mm_p=mm_p, av_p=av_p,
                dram_p=dram_p,
            )
            _body(nc, P, mybir)

    nc.finalize()
    return nc


def kernel(x, Wqkv, bqkv, Wo, bo):
    global _cached_nc, last_result
    import ml_dtypes
    from concourse.bass_utils import run_bass_kernel_spmd

    if _cached_nc is None:
        _cached_nc = _build()
    nc = _cached_nc

    bf16 = ml_dtypes.bfloat16
    x = np.asarray(x, dtype=np.float32)
    wq_b = np.ascontiguousarray(np.asarray(Wqkv, dtype=np.float32).astype(bf16))
    wo_b = np.ascontiguousarray(np.asarray(Wo, dtype=np.float32).astype(bf16))

    # transposed diagonal-chunk causal mask: partition = key i, free = (s, query p)
    i_idx = np.arange(128)[:, None, None]
    s_idx = np.arange(R)[None, :, None]
    p_idx = np.arange(128)[None, None, :]

    in_maps = []
    for core in range(8):
        b, r = divmod(core, R)
        xt = np.ascontiguousarray(x[b].T[:, r::R].astype(bf16))
        masked = (i_idx > p_idx) | ((i_idx == p_idx) & (s_idx > r))
        dm = np.where(masked, np.float32(NEG), np.float32(0.0)).reshape(128, 4, 128)
        dm = np.repeat(dm, 2, axis=1).reshape(128, 1024)
        in_maps.append(
            {"xt": xt, "wqkv": wq_b, "wo": wo_b, "dmask": np.ascontiguousarray(dm)}
        )

    last_result = run_bass_kernel_spmd(nc, in_maps, core_ids=list(range(8)))

    y = np.empty((B, T, C), dtype=np.float32)
    for core in range(8):
        b, r = divmod(core, R)
        y[b, r::R, :] = last_result.results[core]["out"]
    return y



# revision 12
# speedup vs baseline: 1.0025x; 1.0025x over previous
"""Distributed causal multi-head attention (Bass/Tile, 8 TRN2 NeuronCores).

Sharding: core = (batch b, rank r), b = core // 4, r = core % 4.  Rank r
owns query/key rows {g : g % 4 == r} (row-interleaved sequence parallel).
Identical SPMD graph on all cores; rank-dependence lives in input data
(x^T shard + a 0/1 diagonal-mask tensor).

v2 restructure vs baseline:
  - k^T gathered in fp8e4m3 (half the collective bytes), upcast on arrival
  - exp batched into 12 large activations per head-pair (vs 64 small),
    scale=1/sqrt(C) folded into the activation
  - causal diag mask applied as a post-exp 0/1 multiply (DVE, strided)
  - scores row-tiled: both heads of a pair run concurrently in the PE
    array (tile_position rows 0-63 / 64-127)
  - q^T computed lazily per pair to cover the AllGather latency
  - softmax reciprocal batched to one [128,8] DVE op per pair; the
    per-query broadcast runs on the idle GpSimd engine
  - kv-gather DMAs batched (1 ktg + 4 vg DMAs per pair)
"""

import numpy as np

B, T, C, H = 2, 2048, 1024, 16
D = C // H            # 64
R = 4                 # ranks per batch group
TOWN = T // R         # 512 rows owned per core
CC = C // 128         # 8 contraction chunks
PAIRS = H // 2        # 8 head pairs
SCALE = 1.0 / 32.0    # 1/sqrt(C)
KT_ELEMS = C * TOWN   # 524288, k^T shard elems (also v shard elems)

# exp strips: (jj, sb0, nsb); jj = local key-chunk index (l0 = 128*jj),
# sb = owner rank of the key chunk.  Strip = nsb chunks of [128, 512-l0].
STRIPS = [(0, 0, 2), (0, 2, 2), (1, 0, 2), (1, 2, 2), (2, 0, 4), (3, 0, 4)]

_cached_nc = None
last_result = None
_DEBUG = False


def _load_phase(nc, P, mybir):
    from concourse.bass import ts
    F32, BF16 = mybir.dt.float32, mybir.dt.bfloat16

    dmask = P["const_p"].tile([128, 512], BF16, tag="dmask")
    nc.sync.dma_start(dmask[:], P["dmask_ext"][:])
    P["dmask"] = dmask

    xt_sb = P["x_p"].tile([128, CC * TOWN], BF16, tag="xt")
    for cc in range(CC):
        nc.sync.dma_start(xt_sb[:, cc * TOWN : (cc + 1) * TOWN], P["xt_ext"][ts(cc, 128), :])
    P["xt_sb"] = xt_sb

    # k,v weight columns, interleaved per contraction chunk: [k 1024 | v 1024]
    wqkv_kv = P["big_p"].tile([128, CC * 2048], BF16, tag="big")
    for cc in range(CC):
        nc.sync.dma_start(
            wqkv_kv[:, cc * 2048 : cc * 2048 + 1024],
            P["wqkv_ext"][ts(cc, 128), C : 2 * C],
        )
        nc.sync.dma_start(
            wqkv_kv[:, cc * 2048 + 1024 : cc * 2048 + 2048],
            P["wqkv_ext"][ts(cc, 128), 2 * C : 3 * C],
        )
    P["wqkv_kv"] = wqkv_kv
    wqkv_q = P["w_p"].tile([128, CC * C], BF16, tag="wq")
    for cc in range(CC):
        nc.sync.dma_start(wqkv_q[:, cc * C : (cc + 1) * C], P["wqkv_ext"][ts(cc, 128), 0:C])
    P["wqkv_q"] = wqkv_q
    wo_sb = P["w_p"].tile([128, CC * C], BF16, tag="wo")
    for cc in range(CC):
        nc.sync.dma_start(wo_sb[:, cc * C : (cc + 1) * C], P["wo_ext"][ts(cc, 128), :])
    P["wo_sb"] = wo_sb


def _qkv_phase(nc, P, mybir):
    """k^T (fp8) then AllGather#1; v (bf16) then AllGather#2."""
    F32, BF16, F8 = mybir.dt.float32, mybir.dt.bfloat16, mybir.dt.float8e4
    xt_sb, wqkv_kv = P["xt_sb"], P["wqkv_kv"]
    mm_p = P["mm_p"]
    groups = [[0, 1, 2, 3], [4, 5, 6, 7]]

    kt_sb = P["kv_p"].tile([128, CC * TOWN], F8, tag="kt")
    for qc in range(CC):
        ps = mm_p.tile([128, 512], F32, tag="strip")
        for cc in range(CC):
            nc.tensor.matmul(
                ps[:, 0:TOWN],
                wqkv_kv[:, cc * 2048 + qc * 128 : cc * 2048 + (qc + 1) * 128],
                xt_sb[:, cc * TOWN : (cc + 1) * TOWN],
                start=(cc == 0),
                stop=(cc == CC - 1),
            )
        nc.vector.tensor_copy(kt_sb[:, qc * TOWN : (qc + 1) * TOWN], ps[:, 0:TOWN])
    kt_bounce = P["dram_p"].tile([KT_ELEMS], F8, tag="kt_bounce")
    nc.sync.dma_start(
        kt_bounce[:].rearrange("(q p k) -> p q k", p=128, q=CC),
        kt_sb[:].rearrange("p (q k) -> p q k", q=CC),
    )
    kt_gath = P["dram_p"].tile([R * KT_ELEMS], F8, tag="kt_gath")
    nc.gpsimd.collective_compute(
        "AllGather", mybir.AluOpType.bypass, replica_groups=groups,
        ins=[kt_bounce.opt()], outs=[kt_gath.opt()],
    )
    P["kt_gath"] = kt_gath

    v_loc = P["kv_p"].tile([128, 4 * C], BF16, tag="vl")
    for t in range(4):
        for hf in range(2):
            ps = mm_p.tile([128, 512], F32, tag="strip")
            for cc in range(CC):
                nc.tensor.matmul(
                    ps[:, 0:512],
                    xt_sb[:, cc * TOWN + t * 128 : cc * TOWN + (t + 1) * 128],
                    wqkv_kv[:, cc * 2048 + 1024 + hf * 512 : cc * 2048 + 1024 + (hf + 1) * 512],
                    start=(cc == 0),
                    stop=(cc == CC - 1),
                )
            nc.vector.tensor_copy(
                v_loc[:, t * C + hf * 512 : t * C + (hf + 1) * 512], ps[:, 0:512]
            )
    v_bounce = P["dram_p"].tile([TOWN * C], BF16, tag="v_bounce")
    nc.sync.dma_start(
        v_bounce[:].rearrange("(t p c) -> p t c", p=128, t=4),
        v_loc[:].rearrange("p (t c) -> p t c", t=4),
    )
    v_gath = P["dram_p"].tile([R * TOWN * C], BF16, tag="v_gath")
    nc.gpsimd.collective_compute(
        "AllGather", mybir.AluOpType.bypass, replica_groups=groups,
        ins=[v_bounce.opt()], outs=[v_gath.opt()],
    )
    P["v_gath"] = v_gath


def _issue_gathers(nc, P, p, mybir):
    """Prefetch pair p's gathered k^T (fp8) and v (bf16) into SBUF."""
    BF16, F8 = mybir.dt.bfloat16, mybir.dt.float8e4
    ktg8 = P["ktg8_p"].tile([128, 16 * 128], F8, tag="ktg8")
    ksrc = P["kt_gath"][:].rearrange("(sb q k) -> q sb k", sb=R, k=TOWN)[
        p * 128 : (p + 1) * 128, :, :
    ]
    nc.sync.dma_start(ktg8[:].rearrange("q (sb k) -> q sb k", sb=R), ksrc)

    vg = P["vg_p"].tile([128, 16 * 130], BF16, tag="vg")
    nc.vector.memset(vg[:].rearrange("k (s y) -> k s y", y=65)[:, :, 64:65], 1.0)
    for sb in range(R):
        for hh in range(2):
            vsrc = P["v_gath"][sb * TOWN * C : (sb + 1) * TOWN * C].rearrange(
                "(jj k c) -> k jj c", jj=4, c=C
            )[:, :, p * 128 + hh * 64 : p * 128 + (hh + 1) * 64]
            vdst = vg[:, sb * 520 : (sb + 1) * 520].rearrange(
                "k (jj x) -> k jj x", x=130
            )[:, :, hh * 65 : hh * 65 + 64]
            nc.sync.dma_start(vdst, vsrc)
    P[f"ktg8_{p}"] = ktg8
    P[f"vg_{p}"] = vg


def _attention_pair(nc, P, p, mybir):
    F32, BF16 = mybir.dt.float32, mybir.dt.bfloat16
    AFT = mybir.ActivationFunctionType
    mm_p, av_p = P["mm_p"], P["av_p"]
    xt_sb, wqkv_q, qt_sb, dmask = P["xt_sb"], P["wqkv_q"], P["qt_sb"], P["dmask"]

    # lazy q^T chunk for this pair
    ps = mm_p.tile([128, 512], F32, tag="strip")
    for cc in range(CC):
        nc.tensor.matmul(
            ps[:, 0:TOWN],
            wqkv_q[:, cc * C + p * 128 : cc * C + (p + 1) * 128],
            xt_sb[:, cc * TOWN : (cc + 1) * TOWN],
            start=(cc == 0),
            stop=(cc == CC - 1),
        )
    nc.vector.tensor_copy(qt_sb[:, p * TOWN : (p + 1) * TOWN], ps[:, 0:TOWN])

    # upcast this pair's gathered k^T
    ktg = P["ktg_p"].tile([128, 16 * 128], BF16, tag="ktg")
    nc.vector.tensor_copy(ktg[:], P[f"ktg8_{p}"][:])
    vg = P[f"vg_{p}"]

    att2 = P["big_p"].tile([128, 2 * 16 * 512], BF16, tag="big")

    # scores + exp, strip by strip; hh0/hh1 interleaved for PE row-tiling
    for jj, sb0, nsb in STRIPS:
        l0 = jj * 128
        n = 512 - l0
        stride = 512 if jj < 2 else n  # keep each MM output inside one PSUM bank
        strips = []
        for hh in range(2):
            st = mm_p.tile([128, nsb * stride], F32, tag="strip", name="strip")
            strips.append(st)
        for i in range(nsb):
            sb = sb0 + i
            s = sb * 4 + jj
            for hh in range(2):
                nc.tensor.matmul(
                    strips[hh][:, i * stride : i * stride + n],
                    ktg[hh * 64 : (hh + 1) * 64, s * 128 : (s + 1) * 128],
                    qt_sb[hh * 64 : (hh + 1) * 64, p * TOWN + l0 : (p + 1) * TOWN],
                    start=True,
                    stop=True,
                    tile_position=(hh * 64, 0),
                )
        for hh in range(2):
            att2h = att2[:, hh * 8192 : (hh + 1) * 8192].rearrange(
                "q (sb x) -> q sb x", sb=4
            )
            nc.scalar.activation(
                att2h[:, sb0 : sb0 + nsb, jj * 512 + l0 : (jj + 1) * 512],
                strips[hh][:].rearrange("q (s x) -> q s x", x=stride)[:, :, 0:n],
                AFT.Exp,
                scale=SCALE,
            )

    # post-exp 0/1 diagonal mask (one strided DVE mul per (hh, jj))
    for hh in range(2):
        att2h = att2[:, hh * 8192 : (hh + 1) * 8192].rearrange(
            "q (sb x) -> q sb x", sb=4
        )
        dm3 = dmask[:].rearrange("q (sb x) -> q sb x", x=128)
        for jj in range(4):
            l0 = jj * 128
            blk = att2h[:, :, jj * 512 + l0 : jj * 512 + l0 + 128]
            nc.vector.tensor_mul(blk, blk, dm3)

    # AV (ones-row gives the softmax denominator as row 64)
    avs = []
    for hh in range(2):
        avs.append(av_p.tile([65, TOWN], F32, tag="av", name="avs"))
    for s in range(16):
        jj = s % 4
        l0 = jj * 128
        for hh in range(2):
            nc.tensor.matmul(
                avs[hh][:, l0:],
                vg[:, s * 130 + hh * 65 : s * 130 + hh * 65 + 65],
                att2[:, hh * 8192 + s * 512 + l0 : hh * 8192 + (s + 1) * 512],
                start=(s == 0),
                stop=(s == 15),
            )

    # normalize: den rows -> [128,8] reciprocal -> gpsimd broadcast -> mul
    den_sb = P["sm_p"].tile([128, TOWN], F32, tag="den_sb", bufs=2)
    for hh in range(2):
        nc.vector.tensor_copy(den_sb[hh * 64 : hh * 64 + 1, :], avs[hh][64:65, :])
    den_all, den_rec = P["den_all"], P["den_rec"]
    for hh in range(2):
        nc.sync.dma_start(
            den_all[:, p * 8 + hh * 4 : p * 8 + hh * 4 + 4],
            den_sb[hh * 64 : hh * 64 + 1, :],
        )
    nc.vector.reciprocal(den_rec[:, p * 8 : p * 8 + 8], den_all[:, p * 8 : p * 8 + 8])
    recbs = []
    for hh in range(2):
        recb = P["sm_p"].tile([1, TOWN], F32, tag="recb2", bufs=2, name="recb")
        nc.sync.dma_start(
            recb[0:1, :], den_rec[:, p * 8 + hh * 4 : p * 8 + hh * 4 + 4]
        )
        recbs.append(recb)
    outT_sb = P["outT_sb"]
    for hh in range(2):
        bcs = P["sm_p"].tile([64, TOWN], F32, tag="bcs", bufs=2)
        nc.gpsimd.partition_broadcast(bcs[:], recbs[hh][0:1, :])
        nc.vector.tensor_mul(
            outT_sb[hh * 64 : (hh + 1) * 64, p * TOWN : (p + 1) * TOWN],
            avs[hh][0:64, :],
            bcs[:],
        )
    if P.get("dbg_ext") is not None and p == 0:
        nc.sync.dma_start(P["dbg_ext"][:, 0:16384], att2[:])
        nc.sync.dma_start(P["dbg_ext"][:, 16384:16896], qt_sb[:, 0:TOWN])
        nc.sync.dma_start(P["dbg_ext"][:, 16896:18944], ktg[:])
        nc.sync.dma_start(P["dbg_ext"][:, 18944:19456], outT_sb[:, 0:TOWN])
        dbg_den = P["sm_p"].tile([128, TOWN], mybir.dt.bfloat16, tag="dbg_den")
        nc.vector.tensor_copy(dbg_den[:], den_sb[:])
        nc.sync.dma_start(P["dbg_ext"][:, 19456:19968], dbg_den[:])


def _wo_phase(nc, P, mybir):
    F32 = mybir.dt.float32
    outT_sb, wo_sb, mm_p = P["outT_sb"], P["wo_sb"], P["mm_p"]
    y_sb = P["y_p"].tile([128, 4 * C], F32, tag="y")
    for t in range(4):
        for hf in range(2):
            ps = mm_p.tile([128, 512], F32, tag="strip")
            for cc in range(CC):
                nc.tensor.matmul(
                    ps[:, 0:512],
                    outT_sb[:, cc * TOWN + t * 128 : cc * TOWN + (t + 1) * 128],
                    wo_sb[:, cc * C + hf * 512 : cc * C + (hf + 1) * 512],
                    start=(cc == 0),
                    stop=(cc == CC - 1),
                )
            nc.vector.tensor_copy(y_sb[:, t * C + hf * 512 : t * C + (hf + 1) * 512], ps[:, 0:512])
    for t in range(4):
        nc.sync.dma_start(P["out_ext"][t * 128 : (t + 1) * 128, :], y_sb[:, t * C : (t + 1) * C])


def _body(nc, P, mybir):
    F32, BF16 = mybir.dt.float32, mybir.dt.bfloat16
    _load_phase(nc, P, mybir)
    _qkv_phase(nc, P, mybir)

    qt_sb = P["qt_p"].tile([128, CC * TOWN], BF16, tag="qt")
    P["qt_sb"] = qt_sb
    outT_sb = P["outT_p"].tile([128, PAIRS * TOWN], BF16, tag="outT")
    P["outT_sb"] = outT_sb
    P["den_all"] = P["sm_p"].tile([128, 64], F32, tag="den_all", name="den_all")
    P["den_rec"] = P["sm_p"].tile([128, 64], F32, tag="den_rec", name="den_rec")

    _issue_gathers(nc, P, 0, mybir)
    for p in range(PAIRS):
        if p + 1 < PAIRS:
            _issue_gathers(nc, P, p + 1, mybir)
        _attention_pair(nc, P, p, mybir)

    _wo_phase(nc, P, mybir)


def _build():
    import concourse.mybir as mybir
    import concourse.tile as tile
    from concourse import bacc

    F32, BF16 = mybir.dt.float32, mybir.dt.bfloat16

    nc = bacc.Bacc("TRN2", target_bir_lowering=False, debug=False, num_devices=8)
    P = {
        "xt_ext": nc.declare_dram_parameter("xt", [C, TOWN], BF16, isOutput=False),
        "wqkv_ext": nc.declare_dram_parameter("wqkv", [C, 3 * C], BF16, isOutput=False),
        "wo_ext": nc.declare_dram_parameter("wo", [C, C], BF16, isOutput=False),
        "dmask_ext": nc.declare_dram_parameter("dmask", [128, 512], BF16, isOutput=False),
        "out_ext": nc.declare_dram_parameter("out", [TOWN, C], F32, isOutput=True),
    }
    if _DEBUG:
        P["dbg_ext"] = nc.declare_dram_parameter("dbg", [128, 20480], BF16, isOutput=True)

    with tile.TileContext(nc) as tc:
        with (
            tc.tile_pool(name="const", bufs=1) as const_p,
            tc.tile_pool(name="w", bufs=1) as w_p,
            tc.tile_pool(name="big", bufs=2) as big_p,
            tc.tile_pool(name="x", bufs=1) as x_p,
            tc.tile_pool(name="kv", bufs=1) as kv_p,
            tc.tile_pool(name="qt", bufs=1) as qt_p,
            tc.tile_pool(name="ktg8", bufs=2) as ktg8_p,
            tc.tile_pool(name="ktg", bufs=2) as ktg_p,
            tc.tile_pool(name="vg", bufs=2) as vg_p,
            tc.tile_pool(name="outT", bufs=1) as outT_p,
            tc.tile_pool(name="y", bufs=1) as y_p,
            tc.tile_pool(name="sm", bufs=1) as sm_p,
            tc.tile_pool(name="mmp", bufs=3, space="PSUM") as mm_p,
            tc.tile_pool(name="avp", bufs=2, space="PSUM") as av_p,
            tc.tile_pool(name="dram", bufs=1, space="DRAM") as dram_p,
        ):
            P.update(
                const_p=const_p, w_p=w_p, big_p=big_p, x_p=x_p, kv_p=kv_p,
                qt_p=qt_p, ktg8_p=ktg8_p, ktg_p=ktg_p, vg_p=vg_p,
                outT_p=outT_p, y_p=y_p, sm_p=sm_p, mm_p=mm_p, av_p=av_p,
                dram_p=dram_p,
            )
            _body(nc, P, mybir)

    nc.finalize()
    return nc


def kernel(x, Wqkv, bqkv, Wo, bo):
    global _cached_nc, last_result
    import ml_dtypes
    from concourse.bass_utils import run_bass_kernel_spmd

    if _cached_nc is None:
        _cached_nc = _build()
    nc = _cached_nc

    bf16 = ml_dtypes.bfloat16
    x = np.asarray(x, dtype=np.float32)
    wq_b = np.ascontiguousarray(np.asarray(Wqkv, dtype=np.float32).astype(bf16))
    wo_b = np.ascontiguousarray(np.asarray(Wo, dtype=np.float32).astype(bf16))

    # 0/1 diagonal-chunk mask: partition = key m, free = (sb, query i)
    m_idx = np.arange(128)[:, None, None]
    s_idx = np.arange(R)[None, :, None]
    i_idx = np.arange(128)[None, None, :]

    in_maps = []
    for core in range(8):
        b, r = divmod(core, R)
        xt = np.ascontiguousarray(x[b].T[:, r::R].astype(bf16))
        masked = (m_idx > i_idx) | ((m_idx == i_idx) & (s_idx > r))
        dm = np.where(masked, 0.0, 1.0).astype(bf16).reshape(128, 512)
        in_maps.append(
            {"xt": xt, "wqkv": wq_b, "wo": wo_b, "dmask": np.ascontiguousarray(dm)}
        )

    last_result = run_bass_kernel_spmd(nc, in_maps, core_ids=list(range(8)))

    y = np.empty((B, T, C), dtype=np.float32)
    for core in range(8):
        b, r = divmod(core, R)
        y[b, r::R, :] = last_result.results[core]["out"]
    return y


# revision 15
# speedup vs baseline: 1.0670x; 1.0643x over previous
"""Distributed causal multi-head attention (Bass/Tile, 8 TRN2 NeuronCores).

Sharding: core = (batch b, rank r), b = core // 4, r = core % 4.  Rank r
owns query/key rows {g : g % 4 == r} (row-interleaved sequence parallel).
Identical SPMD graph on all cores; rank-dependence lives in input data
(x^T shard + a 0/1 diagonal-mask tensor).

v3 structure:
  - k^T gathered in fp8e4m3 (half bytes), in two half AllGathers (pairs
    0-3 / 4-7) so scoring starts as soon as possible; v in one bf16 AG
  - a tiny warm-up collective absorbs the ~35us first-collective latency
  - exp batched into 12 large activations per head-pair, scale folded in
  - causal diag mask applied as a post-exp 0/1 multiply (DVE, strided)
  - scores row-tiled: both heads of a pair concurrent in the PE array
  - software pipelining: scores/exp phase runs 3 pairs ahead of the
    AV/normalize phase, so the PE always has score work while ACT drains
    exps and the AV phase never head-of-line-blocks the queues
  - q^T computed eagerly during the AllGather window
  - softmax reciprocal batched to one [128,8] DVE op per pair; per-query
    broadcast on the idle GpSimd engine
"""

import numpy as np

B, T, C, H = 2, 2048, 1024, 16
D = C // H            # 64
R = 4                 # ranks per batch group
TOWN = T // R         # 512 rows owned per core
CC = C // 128         # 8 contraction chunks
PAIRS = H // 2        # 8 head pairs
SCALE = 1.0 / 32.0    # 1/sqrt(C)
KT_ELEMS = C * TOWN   # 524288, k^T shard elems (also v shard elems)
HKT = KT_ELEMS // 2   # elems per k^T half (pairs 0-3 or 4-7)

# exp strips: (jj, sb0, nsb); jj = local key-chunk index (l0 = 128*jj),
# sb = owner rank of the key chunk.  Strip = nsb chunks of [128, 512-l0].
STRIPS = [(0, 0, 2), (0, 2, 2), (1, 0, 2), (1, 2, 2), (2, 0, 4), (3, 0, 4)]

_cached_nc = None
last_result = None
_DEBUG = False


def _load_phase(nc, P, mybir):
    from concourse.bass import ts
    F32, BF16 = mybir.dt.float32, mybir.dt.bfloat16

    dmask = P["const_p"].tile([128, 512], BF16, tag="dmask")
    nc.sync.dma_start(dmask[:], P["dmask_ext"][:])
    P["dmask"] = dmask

    xt_sb = P["x_p"].tile([128, CC * TOWN], BF16, tag="xt")
    for cc in range(CC):
        nc.sync.dma_start(xt_sb[:, cc * TOWN : (cc + 1) * TOWN], P["xt_ext"][ts(cc, 128), :])
    P["xt_sb"] = xt_sb

    # k,v weight columns, interleaved per contraction chunk: [k 1024 | v 1024]
    wqkv_kv = P["big_p"].tile([128, CC * 2048], BF16, tag="big")
    for cc in range(CC):
        nc.sync.dma_start(
            wqkv_kv[:, cc * 2048 : cc * 2048 + 1024],
            P["wqkv_ext"][ts(cc, 128), C : 2 * C],
        )
        nc.sync.dma_start(
            wqkv_kv[:, cc * 2048 + 1024 : cc * 2048 + 2048],
            P["wqkv_ext"][ts(cc, 128), 2 * C : 3 * C],
        )
    P["wqkv_kv"] = wqkv_kv


def _load_wq_wo(nc, P, mybir):
    """Deferred: issued after the collectives are triggered."""
    from concourse.bass import ts
    BF16 = mybir.dt.bfloat16
    wqkv_q = P["w_p"].tile([128, CC * C], BF16, tag="wq")
    for cc in range(CC):
        nc.sync.dma_start(wqkv_q[:, cc * C : (cc + 1) * C], P["wqkv_ext"][ts(cc, 128), 0:C])
    P["wqkv_q"] = wqkv_q
    wo_sb = P["w_p"].tile([128, CC * C], BF16, tag="wo")
    for cc in range(CC):
        nc.sync.dma_start(wo_sb[:, cc * C : (cc + 1) * C], P["wo_ext"][ts(cc, 128), :])
    P["wo_sb"] = wo_sb


def _qkv_phase(nc, P, mybir):
    """k^T (fp8) in two half-AllGathers, then v (bf16), then eager q^T."""
    F32, BF16, F8 = mybir.dt.float32, mybir.dt.bfloat16, mybir.dt.float8e4
    xt_sb, wqkv_kv = P["xt_sb"], P["wqkv_kv"]
    mm_p = P["mm_p"]
    groups = [[0, 1, 2, 3], [4, 5, 6, 7]]

    # warm-up: a tiny collective to absorb the first-collective latency
    warm_in = P["dram_p"].tile([512], F32, tag="warm_in")
    warmz = P["const_p"].tile([128, 4], F32, tag="warmz", name="warmz")
    nc.vector.memset(warmz[:], 0.0)
    nc.sync.dma_start(warm_in[:].rearrange("(p f) -> p f", p=128), warmz[:])
    warm_out = P["dram_p"].tile([4 * 512], F32, tag="warm_out")
    nc.gpsimd.collective_compute(
        "AllGather", mybir.AluOpType.bypass, replica_groups=groups,
        ins=[warm_in.opt()], outs=[warm_out.opt()],
    )

    kt_sb = P["y_p"].tile([128, CC * TOWN], F8, tag="y", name="kt_sb")
    for half in range(2):
        for qc in range(4 * half, 4 * (half + 1)):
            ps = mm_p.tile([128, 512], F32, tag="strip")
            for cc in range(CC):
                nc.tensor.matmul(
                    ps[:, 0:TOWN],
                    wqkv_kv[:, cc * 2048 + qc * 128 : cc * 2048 + (qc + 1) * 128],
                    xt_sb[:, cc * TOWN : (cc + 1) * TOWN],
                    start=(cc == 0),
                    stop=(cc == CC - 1),
                )
            nc.vector.tensor_copy(kt_sb[:, qc * TOWN : (qc + 1) * TOWN], ps[:, 0:TOWN])
        kt_bounce = P["dram_p"].tile([HKT], F8, tag=f"kt_bounce{half}")
        nc.sync.dma_start(
            kt_bounce[:].rearrange("(q p k) -> p q k", p=128, q=4),
            kt_sb[:, half * 4 * TOWN : (half + 1) * 4 * TOWN].rearrange(
                "p (q k) -> p q k", q=4
            ),
        )
        kt_gath = P["dram_p"].tile([R * HKT], F8, tag=f"kt_gath{half}")
        nc.gpsimd.collective_compute(
            "AllGather", mybir.AluOpType.bypass, replica_groups=groups,
            ins=[kt_bounce.opt()], outs=[kt_gath.opt()],
        )
        P[f"kt_gath{half}"] = kt_gath

    v_loc = P["kv_p"].tile([128, 4 * C], BF16, tag="vl")
    for t in range(4):
        for hf in range(2):
            ps = mm_p.tile([128, 512], F32, tag="strip")
            for cc in range(CC):
                nc.tensor.matmul(
                    ps[:, 0:512],
                    xt_sb[:, cc * TOWN + t * 128 : cc * TOWN + (t + 1) * 128],
                    wqkv_kv[:, cc * 2048 + 1024 + hf * 512 : cc * 2048 + 1024 + (hf + 1) * 512],
                    start=(cc == 0),
                    stop=(cc == CC - 1),
                )
            nc.vector.tensor_copy(
                v_loc[:, t * C + hf * 512 : t * C + (hf + 1) * 512], ps[:, 0:512]
            )
    v_bounce = P["dram_p"].tile([TOWN * C], BF16, tag="v_bounce")
    nc.sync.dma_start(
        v_bounce[:].rearrange("(t p c) -> p t c", p=128, t=4),
        v_loc[:].rearrange("p (t c) -> p t c", t=4),
    )
    v_gath = P["dram_p"].tile([R * TOWN * C], BF16, tag="v_gath")
    nc.gpsimd.collective_compute(
        "AllGather", mybir.AluOpType.bypass, replica_groups=groups,
        ins=[v_bounce.opt()], outs=[v_gath.opt()],
    )
    P["v_gath"] = v_gath

    # deferred weight loads, then eager q^T (fills the AllGather window)
    _load_wq_wo(nc, P, mybir)
    qt_sb = P["qt_p"].tile([128, CC * TOWN], BF16, tag="qt")
    for p in range(CC):
        ps = mm_p.tile([128, 512], F32, tag="strip")
        for cc in range(CC):
            nc.tensor.matmul(
                ps[:, 0:TOWN],
                P["wqkv_q"][:, cc * C + p * 128 : cc * C + (p + 1) * 128],
                xt_sb[:, cc * TOWN : (cc + 1) * TOWN],
                start=(cc == 0),
                stop=(cc == CC - 1),
            )
        nc.vector.tensor_copy(qt_sb[:, p * TOWN : (p + 1) * TOWN], ps[:, 0:TOWN])
    P["qt_sb"] = qt_sb


def _issue_gathers(nc, P, p, mybir):
    """Prefetch pair p's gathered k^T (fp8) and v (bf16) into SBUF."""
    BF16, F8 = mybir.dt.bfloat16, mybir.dt.float8e4
    ktg8 = P["ktg8_p"].tile([128, 16 * 128], F8, tag="ktg8")
    half, pl = p // 4, p % 4
    ksrc = P[f"kt_gath{half}"][:].rearrange("(sb q k) -> q sb k", sb=R, k=TOWN)[
        pl * 128 : (pl + 1) * 128, :, :
    ]
    nc.sync.dma_start(ktg8[:].rearrange("q (sb k) -> q sb k", sb=R), ksrc)

    vg = P["vg_p"].tile([128, 16 * 130], BF16, tag="vg")
    nc.vector.memset(vg[:].rearrange("k (s y) -> k s y", y=65)[:, :, 64:65], 1.0)
    for sb in range(R):
        for hh in range(2):
            vsrc = P["v_gath"][sb * TOWN * C : (sb + 1) * TOWN * C].rearrange(
                "(jj k c) -> k jj c", jj=4, c=C
            )[:, :, p * 128 + hh * 64 : p * 128 + (hh + 1) * 64]
            vdst = vg[:, sb * 520 : (sb + 1) * 520].rearrange(
                "k (jj x) -> k jj x", x=130
            )[:, :, hh * 65 : hh * 65 + 64]
            nc.sync.dma_start(vdst, vsrc)
    P[f"ktg8_{p}"] = ktg8
    P[f"vg_{p}"] = vg


def _scores_phase(nc, P, p, mybir):
    """q.k^T scores, exp (batched, scaled), post-exp diag mask."""
    F32, BF16 = mybir.dt.float32, mybir.dt.bfloat16
    AFT = mybir.ActivationFunctionType
    mm_p = P["mm_p"]
    qt_sb, dmask = P["qt_sb"], P["dmask"]

    ktg = P["ktg_p"].tile([128, 16 * 128], BF16, tag="ktg")
    nc.vector.tensor_copy(ktg[:], P[f"ktg8_{p}"][:])

    att2 = P["big_p"].tile([128, 2 * 16 * 512], BF16, tag="big")
    P[f"att2_{p}"] = att2

    for jj, sb0, nsb in STRIPS:
        l0 = jj * 128
        n = 512 - l0
        stride = 512 if jj < 2 else n  # keep each MM output inside one PSUM bank
        strips = []
        for hh in range(2):
            st = mm_p.tile([128, nsb * stride], F32, tag="strip", name="strip")
            strips.append(st)
        for i in range(nsb):
            sb = sb0 + i
            s = sb * 4 + jj
            for hh in range(2):
                nc.tensor.matmul(
                    strips[hh][:, i * stride : i * stride + n],
                    ktg[hh * 64 : (hh + 1) * 64, s * 128 : (s + 1) * 128],
                    qt_sb[hh * 64 : (hh + 1) * 64, p * TOWN + l0 : (p + 1) * TOWN],
                    start=True,
                    stop=True,
                    tile_position=(hh * 64, 0),
                )
        for hh in range(2):
            att2h = att2[:, hh * 8192 : (hh + 1) * 8192].rearrange(
                "q (sb x) -> q sb x", sb=4
            )
            nc.scalar.activation(
                att2h[:, sb0 : sb0 + nsb, jj * 512 + l0 : (jj + 1) * 512],
                strips[hh][:].rearrange("q (s x) -> q s x", x=stride)[:, :, 0:n],
                AFT.Exp,
                scale=SCALE,
            )

    for hh in range(2):
        att2h = att2[:, hh * 8192 : (hh + 1) * 8192].rearrange(
            "q (sb x) -> q sb x", sb=4
        )
        dm3 = dmask[:].rearrange("q (sb x) -> q sb x", x=128)
        for jj in range(4):
            l0 = jj * 128
            blk = att2h[:, :, jj * 512 + l0 : jj * 512 + l0 + 128]
            nc.vector.tensor_mul(blk, blk, dm3)


def _av_phase(nc, P, p, mybir):
    """AV matmuls (ones-row denominator), reciprocal, normalize."""
    F32, BF16 = mybir.dt.float32, mybir.dt.bfloat16
    av_p = P["av_p"]
    att2, vg = P[f"att2_{p}"], P[f"vg_{p}"]

    avs = []
    for hh in range(2):
        avs.append(av_p.tile([65, TOWN], F32, tag="av", name="avs"))
    for s in range(16):
        jj = s % 4
        l0 = jj * 128
        for hh in range(2):
            nc.tensor.matmul(
                avs[hh][:, l0:],
                vg[:, s * 130 + hh * 65 : s * 130 + hh * 65 + 65],
                att2[:, hh * 8192 + s * 512 + l0 : hh * 8192 + (s + 1) * 512],
                start=(s == 0),
                stop=(s == 15),
            )

    den_sb = P["sm_p"].tile([128, TOWN], F32, tag="den_sb", bufs=2)
    for hh in range(2):
        nc.vector.tensor_copy(den_sb[hh * 64 : hh * 64 + 1, :], avs[hh][64:65, :])
    den_all, den_rec = P["den_all"], P["den_rec"]
    for hh in range(2):
        nc.sync.dma_start(
            den_all[:, p * 8 + hh * 4 : p * 8 + hh * 4 + 4],
            den_sb[hh * 64 : hh * 64 + 1, :],
        )
    nc.vector.reciprocal(den_rec[:, p * 8 : p * 8 + 8], den_all[:, p * 8 : p * 8 + 8])
    recbs = []
    for hh in range(2):
        recb = P["sm_p"].tile([1, TOWN], F32, tag="recb2", bufs=2, name="recb")
        nc.sync.dma_start(
            recb[0:1, :], den_rec[:, p * 8 + hh * 4 : p * 8 + hh * 4 + 4]
        )
        recbs.append(recb)
    outT_sb = P["outT_sb"]
    for hh in range(2):
        bcs = P["sm_p"].tile([64, TOWN], F32, tag="bcs", bufs=2)
        nc.gpsimd.partition_broadcast(bcs[:], recbs[hh][0:1, :])
        nc.vector.tensor_mul(
            outT_sb[hh * 64 : (hh + 1) * 64, p * TOWN : (p + 1) * TOWN],
            avs[hh][0:64, :],
            bcs[:],
        )
    if P.get("dbg_ext") is not None and p == 0:
        nc.sync.dma_start(P["dbg_ext"][:, 0:16384], att2[:])
        nc.sync.dma_start(P["dbg_ext"][:, 16384:16896], P["qt_sb"][:, 0:TOWN])
        nc.sync.dma_start(P["dbg_ext"][:, 18944:19456], outT_sb[:, 0:TOWN])
        dbg_den = P["sm_p"].tile([128, TOWN], mybir.dt.bfloat16, tag="dbg_den")
        nc.vector.tensor_copy(dbg_den[:], den_sb[:])
        nc.sync.dma_start(P["dbg_ext"][:, 19456:19968], dbg_den[:])


def _wo_phase(nc, P, mybir):
    F32 = mybir.dt.float32
    outT_sb, wo_sb, mm_p = P["outT_sb"], P["wo_sb"], P["mm_p"]
    for t in range(4):
        y_sb = P["y_p"].tile([128, C], F32, tag="y", name="y_sb")
        for hf in range(2):
            ps = mm_p.tile([128, 512], F32, tag="strip")
            for cc in range(CC):
                nc.tensor.matmul(
                    ps[:, 0:512],
                    outT_sb[:, cc * TOWN + t * 128 : cc * TOWN + (t + 1) * 128],
                    wo_sb[:, cc * C + hf * 512 : cc * C + (hf + 1) * 512],
                    start=(cc == 0),
                    stop=(cc == CC - 1),
                )
            nc.vector.tensor_copy(y_sb[:, hf * 512 : (hf + 1) * 512], ps[:, 0:512])
        nc.sync.dma_start(P["out_ext"][t * 128 : (t + 1) * 128, :], y_sb[:])


def _body(nc, P, mybir):
    F32, BF16 = mybir.dt.float32, mybir.dt.bfloat16
    _load_phase(nc, P, mybir)
    _qkv_phase(nc, P, mybir)

    outT_sb = P["kv_p"].tile([128, PAIRS * TOWN], BF16, tag="vl", name="outT_sb")
    P["outT_sb"] = outT_sb
    P["den_all"] = P["sm_p"].tile([128, 64], F32, tag="den_all", name="den_all")
    P["den_rec"] = P["sm_p"].tile([128, 64], F32, tag="den_rec", name="den_rec")

    # software pipeline: gathers 2 ahead, scores 3 ahead of AV
    _issue_gathers(nc, P, 0, mybir)
    _issue_gathers(nc, P, 1, mybir)
    _scores_phase(nc, P, 0, mybir)
    _issue_gathers(nc, P, 2, mybir)
    _scores_phase(nc, P, 1, mybir)
    _issue_gathers(nc, P, 3, mybir)
    _scores_phase(nc, P, 2, mybir)
    for p in range(PAIRS):
        _av_phase(nc, P, p, mybir)
        if p + 4 < PAIRS:
            _issue_gathers(nc, P, p + 4, mybir)
        if p + 3 < PAIRS:
            _scores_phase(nc, P, p + 3, mybir)

    _wo_phase(nc, P, mybir)


def _build():
    import concourse.mybir as mybir
    import concourse.tile as tile
    from concourse import bacc

    F32, BF16 = mybir.dt.float32, mybir.dt.bfloat16

    nc = bacc.Bacc("TRN2", target_bir_lowering=False, debug=False, num_devices=8)
    P = {
        "xt_ext": nc.declare_dram_parameter("xt", [C, TOWN], BF16, isOutput=False),
        "wqkv_ext": nc.declare_dram_parameter("wqkv", [C, 3 * C], BF16, isOutput=False),
        "wo_ext": nc.declare_dram_parameter("wo", [C, C], BF16, isOutput=False),
        "dmask_ext": nc.declare_dram_parameter("dmask", [128, 512], BF16, isOutput=False),
        "out_ext": nc.declare_dram_parameter("out", [TOWN, C], F32, isOutput=True),
    }
    if _DEBUG:
        P["dbg_ext"] = nc.declare_dram_parameter("dbg", [128, 20480], BF16, isOutput=True)

    with tile.TileContext(nc) as tc:
        with (
            tc.tile_pool(name="const", bufs=1) as const_p,
            tc.tile_pool(name="w", bufs=1) as w_p,
            tc.tile_pool(name="big", bufs=3) as big_p,
            tc.tile_pool(name="x", bufs=1) as x_p,
            tc.tile_pool(name="kv", bufs=1) as kv_p,
            tc.tile_pool(name="qt", bufs=1) as qt_p,
            tc.tile_pool(name="ktg8", bufs=2) as ktg8_p,
            tc.tile_pool(name="ktg", bufs=2) as ktg_p,
            tc.tile_pool(name="vg", bufs=3) as vg_p,
            tc.tile_pool(name="y", bufs=2) as y_p,
            tc.tile_pool(name="sm", bufs=1) as sm_p,
            tc.tile_pool(name="mmp", bufs=3, space="PSUM") as mm_p,
            tc.tile_pool(name="avp", bufs=2, space="PSUM") as av_p,
            tc.tile_pool(name="dram", bufs=1, space="DRAM") as dram_p,
        ):
            P.update(
                const_p=const_p, w_p=w_p, big_p=big_p, x_p=x_p, kv_p=kv_p,
                qt_p=qt_p, ktg8_p=ktg8_p, ktg_p=ktg_p, vg_p=vg_p,
                y_p=y_p, sm_p=sm_p, mm_p=mm_p, av_p=av_p,
                dram_p=dram_p,
            )
            _body(nc, P, mybir)

    nc.finalize()
    return nc


def kernel(x, Wqkv, bqkv, Wo, bo):
    global _cached_nc, last_result
    import ml_dtypes
    from concourse.bass_utils import run_bass_kernel_spmd

    if _cached_nc is None:
        _cached_nc = _build()
    nc = _cached_nc

    bf16 = ml_dtypes.bfloat16
    x = np.asarray(x, dtype=np.float32)
    wq_b = np.ascontiguousarray(np.asarray(Wqkv, dtype=np.float32).astype(bf16))
    wo_b = np.ascontiguousarray(np.asarray(Wo, dtype=np.float32).astype(bf16))

    # 0/1 diagonal-chunk mask: partition = key m, free = (sb, query i)
    m_idx = np.arange(128)[:, None, None]
    s_idx = np.arange(R)[None, :, None]
    i_idx = np.arange(128)[None, None, :]

    in_maps = []
    for core in range(8):
        b, r = divmod(core, R)
        xt = np.ascontiguousarray(x[b].T[:, r::R].astype(bf16))
        masked = (m_idx > i_idx) | ((m_idx == i_idx) & (s_idx > r))
        dm = np.where(masked, 0.0, 1.0).astype(bf16).reshape(128, 512)
        in_maps.append(
            {"xt": xt, "wqkv": wq_b, "wo": wo_b, "dmask": np.ascontiguousarray(dm)}
        )

    last_result = run_bass_kernel_spmd(nc, in_maps, core_ids=list(range(8)))

    y = np.empty((B, T, C), dtype=np.float32)
    for core in range(8):
        b, r = divmod(core, R)
        y[b, r::R, :] = last_result.results[core]["out"]
    return y


# revision 16
# speedup vs baseline: 1.1014x; 1.0322x over previous
"""Distributed causal multi-head attention (Bass/Tile, 8 TRN2 NeuronCores).

Sharding: core = (batch b, rank r), b = core // 4, r = core % 4.  Rank r
owns query/key rows {g : g % 4 == r} (row-interleaved sequence parallel).
Identical SPMD graph on all cores; rank-dependence lives in input data
(x^T shard + a 0/1 diagonal-mask tensor).

v3 structure:
  - k^T gathered in fp8e4m3 (half bytes), in two half AllGathers (pairs
    0-3 / 4-7) so scoring starts as soon as possible; v in one bf16 AG
  - a tiny warm-up collective absorbs the ~35us first-collective latency
  - exp batched into 12 large activations per head-pair, scale folded in
  - causal diag mask applied as a post-exp 0/1 multiply (DVE, strided)
  - scores row-tiled: both heads of a pair concurrent in the PE array
  - software pipelining: scores/exp phase runs 3 pairs ahead of the
    AV/normalize phase, so the PE always has score work while ACT drains
    exps and the AV phase never head-of-line-blocks the queues
  - q^T computed eagerly during the AllGather window
  - softmax reciprocal batched to one [128,8] DVE op per pair; per-query
    broadcast on the idle GpSimd engine
"""

import numpy as np

B, T, C, H = 2, 2048, 1024, 16
D = C // H            # 64
R = 4                 # ranks per batch group
TOWN = T // R         # 512 rows owned per core
CC = C // 128         # 8 contraction chunks
PAIRS = H // 2        # 8 head pairs
SCALE = 1.0 / 32.0    # 1/sqrt(C)
KT_ELEMS = C * TOWN   # 524288, k^T shard elems (also v shard elems)
HKT = KT_ELEMS // 2   # elems per k^T half (pairs 0-3 or 4-7)

# exp strips: (jj, sb0, nsb); jj = local key-chunk index (l0 = 128*jj),
# sb = owner rank of the key chunk.  Strip = nsb chunks of [128, 512-l0].
STRIPS = [(0, 0, 2), (0, 2, 2), (1, 0, 2), (1, 2, 2), (2, 0, 4), (3, 0, 4)]

_cached_nc = None
last_result = None
_DEBUG = False


def _load_phase(nc, P, mybir):
    from concourse.bass import ts
    F32, BF16 = mybir.dt.float32, mybir.dt.bfloat16
    groups = [[0, 1, 2, 3], [4, 5, 6, 7]]

    # tiny collective issued before anything else: absorbs the ~35us
    # first-collective ramp so the kt AllGather streams immediately
    warm_in = P["dram_p"].tile([512], F32, tag="warm_in")
    warmz = P["const_p"].tile([128, 4], F32, tag="warmz", name="warmz")
    nc.vector.memset(warmz[:], 0.0)
    nc.sync.dma_start(warm_in[:].rearrange("(p f) -> p f", p=128), warmz[:])
    warm_out = P["dram_p"].tile([4 * 512], F32, tag="warm_out")
    nc.gpsimd.collective_compute(
        "AllGather", mybir.AluOpType.bypass, replica_groups=groups,
        ins=[warm_in.opt()], outs=[warm_out.opt()],
    )

    # dummy matmuls on const data while input DMAs stream: flips the PE
    # HAM throttle to full clock before the real QKV burst
    wwarm = P["const_p"].tile([128, 640], BF16, tag="wwarm", name="wwarm")
    nc.vector.memset(wwarm[:], 0.0)
    for i in range(16):
        ps = P["mm_p"].tile([128, 512], F32, tag="strip", name="ps_warm")
        nc.tensor.matmul(ps[:, 0:512], wwarm[:, 0:128], wwarm[:, 128:640],
                         start=True, stop=True)

    dmask = P["const_p"].tile([128, 512], BF16, tag="dmask")
    nc.sync.dma_start(dmask[:], P["dmask_ext"][:])
    P["dmask"] = dmask

    xt_sb = P["x_p"].tile([128, CC * TOWN], BF16, tag="xt")
    for cc in range(CC):
        nc.sync.dma_start(xt_sb[:, cc * TOWN : (cc + 1) * TOWN], P["xt_ext"][ts(cc, 128), :])
    P["xt_sb"] = xt_sb

    # k,v weight columns, interleaved per contraction chunk: [k 1024 | v 1024]
    wqkv_kv = P["big_p"].tile([128, CC * 2048], BF16, tag="big")
    for cc in range(CC):
        nc.sync.dma_start(
            wqkv_kv[:, cc * 2048 : cc * 2048 + 1024],
            P["wqkv_ext"][ts(cc, 128), C : 2 * C],
        )
        nc.sync.dma_start(
            wqkv_kv[:, cc * 2048 + 1024 : cc * 2048 + 2048],
            P["wqkv_ext"][ts(cc, 128), 2 * C : 3 * C],
        )
    P["wqkv_kv"] = wqkv_kv


def _load_wq_wo(nc, P, mybir):
    """Deferred: issued after the collectives are triggered."""
    from concourse.bass import ts
    BF16 = mybir.dt.bfloat16
    wqkv_q = P["w_p"].tile([128, CC * C], BF16, tag="wq")
    for cc in range(CC):
        nc.sync.dma_start(wqkv_q[:, cc * C : (cc + 1) * C], P["wqkv_ext"][ts(cc, 128), 0:C])
    P["wqkv_q"] = wqkv_q
    wo_sb = P["w_p"].tile([128, CC * C], BF16, tag="wo")
    for cc in range(CC):
        nc.sync.dma_start(wo_sb[:, cc * C : (cc + 1) * C], P["wo_ext"][ts(cc, 128), :])
    P["wo_sb"] = wo_sb


def _qkv_phase(nc, P, mybir):
    """k^T (fp8) in two half-AllGathers, then v (bf16), then eager q^T."""
    F32, BF16, F8 = mybir.dt.float32, mybir.dt.bfloat16, mybir.dt.float8e4
    xt_sb, wqkv_kv = P["xt_sb"], P["wqkv_kv"]
    mm_p = P["mm_p"]
    groups = [[0, 1, 2, 3], [4, 5, 6, 7]]

    kt_sb = P["y_p"].tile([128, CC * TOWN], F8, tag="y", name="kt_sb")
    for half in range(2):
        for qc in range(4 * half, 4 * (half + 1)):
            ps = mm_p.tile([128, 512], F32, tag="strip")
            for cc in range(CC):
                nc.tensor.matmul(
                    ps[:, 0:TOWN],
                    wqkv_kv[:, cc * 2048 + qc * 128 : cc * 2048 + (qc + 1) * 128],
                    xt_sb[:, cc * TOWN : (cc + 1) * TOWN],
                    start=(cc == 0),
                    stop=(cc == CC - 1),
                )
            nc.vector.tensor_copy(kt_sb[:, qc * TOWN : (qc + 1) * TOWN], ps[:, 0:TOWN])
        kt_bounce = P["dram_p"].tile([HKT], F8, tag=f"kt_bounce{half}")
        nc.sync.dma_start(
            kt_bounce[:].rearrange("(q p k) -> p q k", p=128, q=4),
            kt_sb[:, half * 4 * TOWN : (half + 1) * 4 * TOWN].rearrange(
                "p (q k) -> p q k", q=4
            ),
        )
        kt_gath = P["dram_p"].tile([R * HKT], F8, tag=f"kt_gath{half}")
        nc.gpsimd.collective_compute(
            "AllGather", mybir.AluOpType.bypass, replica_groups=groups,
            ins=[kt_bounce.opt()], outs=[kt_gath.opt()],
        )
        P[f"kt_gath{half}"] = kt_gath

    v_loc = P["kv_p"].tile([128, 4 * C], BF16, tag="vl")
    for t in range(4):
        for hf in range(2):
            ps = mm_p.tile([128, 512], F32, tag="strip")
            for cc in range(CC):
                nc.tensor.matmul(
                    ps[:, 0:512],
                    xt_sb[:, cc * TOWN + t * 128 : cc * TOWN + (t + 1) * 128],
                    wqkv_kv[:, cc * 2048 + 1024 + hf * 512 : cc * 2048 + 1024 + (hf + 1) * 512],
                    start=(cc == 0),
                    stop=(cc == CC - 1),
                )
            nc.vector.tensor_copy(
                v_loc[:, t * C + hf * 512 : t * C + (hf + 1) * 512], ps[:, 0:512]
            )
    v_bounce = P["dram_p"].tile([TOWN * C], BF16, tag="v_bounce")
    nc.sync.dma_start(
        v_bounce[:].rearrange("(t p c) -> p t c", p=128, t=4),
        v_loc[:].rearrange("p (t c) -> p t c", t=4),
    )
    v_gath = P["dram_p"].tile([R * TOWN * C], BF16, tag="v_gath")
    nc.gpsimd.collective_compute(
        "AllGather", mybir.AluOpType.bypass, replica_groups=groups,
        ins=[v_bounce.opt()], outs=[v_gath.opt()],
    )
    P["v_gath"] = v_gath

    # deferred weight loads, then eager q^T (fills the AllGather window)
    _load_wq_wo(nc, P, mybir)
    qt_sb = P["qt_p"].tile([128, CC * TOWN], F8, tag="qt")
    for p in range(CC):
        ps = mm_p.tile([128, 512], F32, tag="strip")
        for cc in range(CC):
            nc.tensor.matmul(
                ps[:, 0:TOWN],
                P["wqkv_q"][:, cc * C + p * 128 : cc * C + (p + 1) * 128],
                xt_sb[:, cc * TOWN : (cc + 1) * TOWN],
                start=(cc == 0),
                stop=(cc == CC - 1),
            )
        nc.vector.tensor_copy(qt_sb[:, p * TOWN : (p + 1) * TOWN], ps[:, 0:TOWN])
    P["qt_sb"] = qt_sb


def _issue_gathers(nc, P, p, mybir):
    """Prefetch pair p's gathered k^T (fp8) and v (bf16) into SBUF."""
    BF16, F8 = mybir.dt.bfloat16, mybir.dt.float8e4
    ktg8 = P["ktg8_p"].tile([128, 16 * 128], F8, tag="ktg8")
    half, pl = p // 4, p % 4
    ksrc = P[f"kt_gath{half}"][:].rearrange("(sb q k) -> q sb k", sb=R, k=TOWN)[
        pl * 128 : (pl + 1) * 128, :, :
    ]
    nc.sync.dma_start(ktg8[:].rearrange("q (sb k) -> q sb k", sb=R), ksrc)

    vg = P["vg_p"].tile([128, 16 * 130], BF16, tag="vg")
    nc.vector.memset(vg[:].rearrange("k (s y) -> k s y", y=65)[:, :, 64:65], 1.0)
    for sb in range(R):
        for hh in range(2):
            vsrc = P["v_gath"][sb * TOWN * C : (sb + 1) * TOWN * C].rearrange(
                "(jj k c) -> k jj c", jj=4, c=C
            )[:, :, p * 128 + hh * 64 : p * 128 + (hh + 1) * 64]
            vdst = vg[:, sb * 520 : (sb + 1) * 520].rearrange(
                "k (jj x) -> k jj x", x=130
            )[:, :, hh * 65 : hh * 65 + 64]
            nc.sync.dma_start(vdst, vsrc)
    P[f"ktg8_{p}"] = ktg8
    P[f"vg_{p}"] = vg


def _scores_phase(nc, P, p, mybir):
    """q.k^T scores, exp (batched, scaled), post-exp diag mask."""
    F32, BF16, F8 = mybir.dt.float32, mybir.dt.bfloat16, mybir.dt.float8e4
    AFT = mybir.ActivationFunctionType
    mm_p = P["mm_p"]
    qt_sb, dmask = P["qt_sb"], P["dmask"]

    ktg = P[f"ktg8_{p}"]

    att2 = P["big_p"].tile([128, 2 * 16 * 512], BF16, tag="big")
    P[f"att2_{p}"] = att2

    for jj, sb0, nsb in STRIPS:
        l0 = jj * 128
        n = 512 - l0
        stride = 512 if jj < 2 else n  # keep each MM output inside one PSUM bank
        strips = []
        for hh in range(2):
            st = mm_p.tile([128, nsb * stride], F32, tag="strip", name="strip")
            strips.append(st)
        for i in range(nsb):
            sb = sb0 + i
            s = sb * 4 + jj
            for hh in range(2):
                nc.tensor.matmul(
                    strips[hh][:, i * stride : i * stride + n],
                    ktg[hh * 64 : (hh + 1) * 64, s * 128 : (s + 1) * 128],
                    qt_sb[hh * 64 : (hh + 1) * 64, p * TOWN + l0 : (p + 1) * TOWN],
                    start=True,
                    stop=True,
                    tile_position=(hh * 64, 0),
                )
        for hh in range(2):
            att2h = att2[:, hh * 8192 : (hh + 1) * 8192].rearrange(
                "q (sb x) -> q sb x", sb=4
            )
            nc.scalar.activation(
                att2h[:, sb0 : sb0 + nsb, jj * 512 + l0 : (jj + 1) * 512],
                strips[hh][:].rearrange("q (s x) -> q s x", x=stride)[:, :, 0:n],
                AFT.Exp,
                scale=SCALE,
            )

    for hh in range(2):
        att2h = att2[:, hh * 8192 : (hh + 1) * 8192].rearrange(
            "q (sb x) -> q sb x", sb=4
        )
        dm3 = dmask[:].rearrange("q (sb x) -> q sb x", x=128)
        for jj in range(4):
            l0 = jj * 128
            blk = att2h[:, :, jj * 512 + l0 : jj * 512 + l0 + 128]
            nc.vector.tensor_mul(blk, blk, dm3)


def _av_phase(nc, P, p, mybir):
    """AV matmuls (ones-row denominator), reciprocal, normalize."""
    F32, BF16 = mybir.dt.float32, mybir.dt.bfloat16
    av_p = P["av_p"]
    att2, vg = P[f"att2_{p}"], P[f"vg_{p}"]

    avs = []
    for hh in range(2):
        avs.append(av_p.tile([65, TOWN], F32, tag="av", name="avs"))
    for s in range(16):
        jj = s % 4
        l0 = jj * 128
        for hh in range(2):
            nc.tensor.matmul(
                avs[hh][:, l0:],
                vg[:, s * 130 + hh * 65 : s * 130 + hh * 65 + 65],
                att2[:, hh * 8192 + s * 512 + l0 : hh * 8192 + (s + 1) * 512],
                start=(s == 0),
                stop=(s == 15),
            )

    den_sb = P["sm_p"].tile([128, TOWN], F32, tag="den_sb", bufs=2)
    for hh in range(2):
        nc.vector.tensor_copy(den_sb[hh * 64 : hh * 64 + 1, :], avs[hh][64:65, :])
    den_all, den_rec = P["den_all"], P["den_rec"]
    for hh in range(2):
        nc.sync.dma_start(
            den_all[:, p * 8 + hh * 4 : p * 8 + hh * 4 + 4],
            den_sb[hh * 64 : hh * 64 + 1, :],
        )
    nc.vector.reciprocal(den_rec[:, p * 8 : p * 8 + 8], den_all[:, p * 8 : p * 8 + 8])
    recbs = []
    for hh in range(2):
        recb = P["sm_p"].tile([1, TOWN], F32, tag="recb2", bufs=2, name="recb")
        nc.sync.dma_start(
            recb[0:1, :], den_rec[:, p * 8 + hh * 4 : p * 8 + hh * 4 + 4]
        )
        recbs.append(recb)
    outT_sb = P["outT_sb"]
    for hh in range(2):
        bcs = P["sm_p"].tile([64, TOWN], F32, tag="bcs", bufs=2)
        nc.gpsimd.partition_broadcast(bcs[:], recbs[hh][0:1, :])
        nc.vector.tensor_mul(
            outT_sb[hh * 64 : (hh + 1) * 64, p * TOWN : (p + 1) * TOWN],
            avs[hh][0:64, :],
            bcs[:],
        )


def _wo_phase(nc, P, mybir):
    F32 = mybir.dt.float32
    outT_sb, wo_sb, mm_p = P["outT_sb"], P["wo_sb"], P["mm_p"]
    for t in range(4):
        y_sb = P["y_p"].tile([128, C], F32, tag="y", name="y_sb")
        for hf in range(2):
            ps = mm_p.tile([128, 512], F32, tag="strip")
            for cc in range(CC):
                nc.tensor.matmul(
                    ps[:, 0:512],
                    outT_sb[:, cc * TOWN + t * 128 : cc * TOWN + (t + 1) * 128],
                    wo_sb[:, cc * C + hf * 512 : cc * C + (hf + 1) * 512],
                    start=(cc == 0),
                    stop=(cc == CC - 1),
                )
            nc.vector.tensor_copy(y_sb[:, hf * 512 : (hf + 1) * 512], ps[:, 0:512])
        nc.sync.dma_start(P["out_ext"][t * 128 : (t + 1) * 128, :], y_sb[:])


def _body(nc, P, mybir):
    F32, BF16 = mybir.dt.float32, mybir.dt.bfloat16
    _load_phase(nc, P, mybir)
    _qkv_phase(nc, P, mybir)

    outT_sb = P["kv_p"].tile([128, PAIRS * TOWN], BF16, tag="vl", name="outT_sb")
    P["outT_sb"] = outT_sb
    P["den_all"] = P["sm_p"].tile([128, 64], F32, tag="den_all", name="den_all")
    P["den_rec"] = P["sm_p"].tile([128, 64], F32, tag="den_rec", name="den_rec")

    # software pipeline: gathers 2 ahead, scores 3 ahead of AV
    _issue_gathers(nc, P, 0, mybir)
    _issue_gathers(nc, P, 1, mybir)
    _scores_phase(nc, P, 0, mybir)
    _issue_gathers(nc, P, 2, mybir)
    _scores_phase(nc, P, 1, mybir)
    _issue_gathers(nc, P, 3, mybir)
    _scores_phase(nc, P, 2, mybir)
    for p in range(PAIRS):
        _av_phase(nc, P, p, mybir)
        if p + 4 < PAIRS:
            _issue_gathers(nc, P, p + 4, mybir)
        if p + 3 < PAIRS:
            _scores_phase(nc, P, p + 3, mybir)

    _wo_phase(nc, P, mybir)


def _build():
    import concourse.mybir as mybir
    import concourse.tile as tile
    from concourse import bacc

    F32, BF16 = mybir.dt.float32, mybir.dt.bfloat16

    nc = bacc.Bacc("TRN2", target_bir_lowering=False, debug=False, num_devices=8)
    P = {
        "xt_ext": nc.declare_dram_parameter("xt", [C, TOWN], BF16, isOutput=False),
        "wqkv_ext": nc.declare_dram_parameter("wqkv", [C, 3 * C], BF16, isOutput=False),
        "wo_ext": nc.declare_dram_parameter("wo", [C, C], BF16, isOutput=False),
        "dmask_ext": nc.declare_dram_parameter("dmask", [128, 512], BF16, isOutput=False),
        "out_ext": nc.declare_dram_parameter("out", [TOWN, C], F32, isOutput=True),
    }
    if _DEBUG:
        P["dbg_ext"] = nc.declare_dram_parameter("dbg", [128, 20480], BF16, isOutput=True)

    with tile.TileContext(nc) as tc:
        with (
            tc.tile_pool(name="const", bufs=1) as const_p,
            tc.tile_pool(name="w", bufs=1) as w_p,
            tc.tile_pool(name="big", bufs=3) as big_p,
            tc.tile_pool(name="x", bufs=1) as x_p,
            tc.tile_pool(name="kv", bufs=1) as kv_p,
            tc.tile_pool(name="qt", bufs=1) as qt_p,
            tc.tile_pool(name="ktg8", bufs=3) as ktg8_p,
            tc.tile_pool(name="vg", bufs=4) as vg_p,
            tc.tile_pool(name="y", bufs=2) as y_p,
            tc.tile_pool(name="sm", bufs=1) as sm_p,
            tc.tile_pool(name="mmp", bufs=3, space="PSUM") as mm_p,
            tc.tile_pool(name="avp", bufs=2, space="PSUM") as av_p,
            tc.tile_pool(name="dram", bufs=1, space="DRAM") as dram_p,
        ):
            P.update(
                const_p=const_p, w_p=w_p, big_p=big_p, x_p=x_p, kv_p=kv_p,
                qt_p=qt_p, ktg8_p=ktg8_p, vg_p=vg_p,
                y_p=y_p, sm_p=sm_p, mm_p=mm_p, av_p=av_p,
                dram_p=dram_p,
            )
            _body(nc, P, mybir)

    nc.finalize()
    return nc


def kernel(x, Wqkv, bqkv, Wo, bo):
    global _cached_nc, last_result
    import ml_dtypes
    from concourse.bass_utils import run_bass_kernel_spmd

    if _cached_nc is None:
        _cached_nc = _build()
    nc = _cached_nc

    bf16 = ml_dtypes.bfloat16
    x = np.asarray(x, dtype=np.float32)
    wq_b = np.ascontiguousarray(np.asarray(Wqkv, dtype=np.float32).astype(bf16))
    wo_b = np.ascontiguousarray(np.asarray(Wo, dtype=np.float32).astype(bf16))

    # 0/1 diagonal-chunk mask: partition = key m, free = (sb, query i)
    m_idx = np.arange(128)[:, None, None]
    s_idx = np.arange(R)[None, :, None]
    i_idx = np.arange(128)[None, None, :]

    in_maps = []
    for core in range(8):
        b, r = divmod(core, R)
        xt = np.ascontiguousarray(x[b].T[:, r::R].astype(bf16))
        masked = (m_idx > i_idx) | ((m_idx == i_idx) & (s_idx > r))
        dm = np.where(masked, 0.0, 1.0).astype(bf16).reshape(128, 512)
        in_maps.append(
            {"xt": xt, "wqkv": wq_b, "wo": wo_b, "dmask": np.ascontiguousarray(dm)}
        )

    last_result = run_bass_kernel_spmd(nc, in_maps, core_ids=list(range(8)))

    y = np.empty((B, T, C), dtype=np.float32)
    for core in range(8):
        b, r = divmod(core, R)
        y[b, r::R, :] = last_result.results[core]["out"]
    return y
